# revision 28
# baseline (speedup 1.0000x reference)
"""GAT model kernel for Trainium2 — 8-core data-parallel over batch.

Per graph: 8-head GAT layer + single-head output attention + LayerNorm +
mean/max pooling. Attention exp uses the exact rank-1 factorization
  exp(leaky_relu(f1_i + f2_j)) = max(e^{f1+f2}, e^{a*(f1+f2)})
plus softmax row-scale invariance (divide row i by e^{a*f1_i}):
  p'[j,i] = mask[i,j] * max(w_i * v1_j, v2_j),
  w = e^{0.98 f1}, v1 = e^{f2}, v2 = e^{0.02 f2}
so only O(N) exps are needed per head instead of O(N^2).
Host side pre-packs weights to bf16 and pre-transposes x / support.
"""

import sys

sys.path.insert(0, "/opt/trn_rl_repo")

import ml_dtypes
import numpy as np

import concourse.bass as bass
from concourse import bacc
import concourse.mybir as mybir
import concourse.tile as tile
from concourse.bass_utils import run_bass_kernel_spmd

AL = mybir.AluOpType
FP = mybir.dt.float32
BF = mybir.dt.bfloat16
EXP = mybir.ActivationFunctionType.Exp
LN_F = mybir.ActivationFunctionType.Ln
RELU = mybir.ActivationFunctionType.Relu
BF_NP = ml_dtypes.bfloat16

B, N, F = 32, 512, 300
H, D = 8, 64
HID = H * D  # 512
OUT = 256
NCORES = 8
GPC = B // NCORES  # graphs per core
P = 128
NCH = N // P  # 4
FC = 3  # ceil(300/128)
ALPHA = 0.02
EPS = 1e-5


# ---- custom fused DVE op: out = max(in0*s0, s1) * in1 ----
from concourse import dve_ops as _dvo
from concourse.dve_spec import Spec as _Spec, Src0 as _S0, Src1 as _S1, C0 as _C0, C1 as _C1, maxx as _maxx, lower as _lower, _has_src1 as _has_src1
from concourse.dve_uop import DveOpSpec as _DveOpSpec


def _register_pmask():
    if "PMASK_ANT" in _dvo._SUB_OPCODE_FOR_NAME:
        return _dvo.OPS[_dvo._SUB_OPCODE_FOR_NAME["PMASK_ANT"] - _dvo._CUSTOM_DVE_ROW_BASE]
    spec = _Spec(
        body=_maxx(_S0 * _C0, _C1) * _S1,
        reference=lambda in0, in1, s0, s1, imm2: (
            np.maximum(in0.astype(np.float32) * s0, s1) * in1
        ).astype(np.float32),
    )
    shas = {}
    opcode = _dvo._CUSTOM_DVE_ROW_BASE + len(_dvo.OPS)
    for ver in ("v3", "v4"):
        r = _DveOpSpec(name="PMASK_ANT", opcode=opcode, uops=_lower(spec, ver=ver),
                       rd1_en=_has_src1(spec))
        shas[ver] = r.sha(ver)
    op = _dvo.DveOp("PMASK_ANT", spec, subdim=False, uops_sha=shas)
    _dvo.OPS.append(op)
    _dvo._SUB_OPCODE_FOR_NAME["PMASK_ANT"] = opcode
    _dvo.CUSTOM_DVE_SPECS["PMASK_ANT"] = spec
    return op


PMASK = _register_pmask()


def build_kernel():
    nc = bacc.Bacc()
    xT_d = nc.dram_tensor("xT_sh", [GPC, P, FC, N], BF, kind="ExternalInput").ap()
    mT_d = nc.dram_tensor("mT_sh", [GPC, P, NCH, N], BF, kind="ExternalInput").ap()
    w1_d = nc.dram_tensor("w1b", [P, FC, HID], BF, kind="ExternalInput").ap()
    a1_d = nc.dram_tensor("a1b", [P, 4, 2 * H], BF, kind="ExternalInput").ap()
    w2_d = nc.dram_tensor("w2b", [P, 4, OUT], BF, kind="ExternalInput").ap()
    a2_d = nc.dram_tensor("a2b", [P, 2, 2], BF, kind="ExternalInput").ap()
    gam_d = nc.dram_tensor("gamb", [P, OUT], FP, kind="ExternalInput").ap()
    b2_d = nc.dram_tensor("b2c", [P, 2], FP, kind="ExternalInput").ap()
    out_d = nc.dram_tensor("out_sh", [GPC, OUT], FP, kind="ExternalOutput").ap()

    ident_d = nc.inline_tensor(np.eye(P, dtype=np.float32).astype(BF_NP), name="idbf").ap()
    sc1_np = np.where(np.arange(2 * H) < H, 1.0 - ALPHA, 1.0).astype(np.float32)
    sc1_d = nc.inline_tensor(sc1_np[:, None], name="sc1c").ap()
    sc2_d = nc.inline_tensor(np.array([[1.0 - ALPHA], [1.0]], dtype=np.float32), name="sc2c").ap()

    with tile.TileContext(nc) as tc:
        cp = tc.alloc_tile_pool(name="const", bufs=1)
        gio = tc.alloc_tile_pool(name="gio", bufs=4)
        wrp = tc.alloc_tile_pool(name="wrp", bufs=2 * H)
        sg = tc.alloc_tile_pool(name="graph", bufs=2)
        sh = tc.alloc_tile_pool(name="head", bufs=3)
        pbig = tc.alloc_tile_pool(name="pbig", bufs=2, space="PSUM")
        pmed = tc.alloc_tile_pool(name="pmed", bufs=2, space="PSUM")
        ptp = tc.alloc_tile_pool(name="ptp", bufs=2, space="PSUM")
        psml = tc.alloc_tile_pool(name="psml", bufs=2, space="PSUM")

        # ---- constants (single clean DMAs) ----
        ident = cp.tile([P, P], BF, tag="ident")
        nc.sync.dma_start(ident[:], ident_d[:, :])
        ones_row = cp.tile([1, P], BF, tag="ones_row")
        nc.vector.memset(ones_row[:], 1.0)
        epst = cp.tile([P, 1], FP, tag="epst")
        nc.vector.memset(epst[:], EPS)
        w1b = cp.tile([P, FC, HID], BF, tag="w1b")
        nc.sync.dma_start(w1b[:], w1_d[:])
        a1b = cp.tile([P, 4, 2 * H], BF, tag="a1b")
        nc.sync.dma_start(a1b[:], a1_d[:])
        w2b = cp.tile([P, 4, OUT], BF, tag="w2b")
        nc.sync.dma_start(w2b[:], w2_d[:])
        a2b = cp.tile([P, 2, 2], BF, tag="a2b")
        nc.sync.dma_start(a2b[:], a2_d[:])
        gamb = cp.tile([P, OUT], FP, tag="gamb")
        nc.sync.dma_start(gamb[:], gam_d[:])
        b2c = cp.tile([P, 2], FP, tag="b2c")
        nc.sync.dma_start(b2c[:], b2_d[:])
        sc1 = cp.tile([2 * H, 1], FP, tag="sc1")
        nc.sync.dma_start(sc1[:], sc1_d[:, :])
        sc2 = cp.tile([2, 1], FP, tag="sc2")
        nc.sync.dma_start(sc2[:], sc2_d[:, :])

        for g in range(GPC):
            xT = gio.tile([P, FC, N], BF, tag="xT")
            nc.sync.dma_start(xT[:], xT_d[g])
            maskT = gio.tile([P, NCH, N], BF, tag="maskT")
            nc.sync.dma_start(maskT[:], mT_d[g])

            # ---- WhT_all [128, 4, 512] bf16 (rows = hid = h*64+d) ----
            whT = sg.tile([P, NCH, N], BF, tag="whT")
            for mc in range(NCH):
                pw = pbig.tile([P, N], FP, tag="b_mm")
                for kc in range(FC):
                    nc.tensor.matmul(
                        pw[:],
                        w1b[:, kc, mc * P : mc * P + P],
                        xT[:, kc, :],
                        start=(kc == 0),
                        stop=(kc == FC - 1),
                    )
                nc.scalar.copy(whT[:, mc, :], pw[:])

            # ---- f rows [16, 512]: 0-7 f1 per head, 8-15 f2 ----
            pf = psml.tile([2 * H, N], FP, tag="s")
            for c in range(NCH):
                nc.tensor.matmul(
                    pf[:], a1b[:, c, :], whT[:, c, :],
                    start=(c == 0), stop=(c == NCH - 1),
                )
            uv1 = sg.tile([2 * H, N], BF, tag="uv1")
            nc.scalar.activation(uv1[:], pf[:], EXP, scale=sc1[:])
            uv2 = sg.tile([2 * H, N], BF, tag="uv2")
            nc.scalar.activation(uv2[:], pf[:], EXP, scale=ALPHA)

            # ---- v columns via transpose: [128, 4, 16] ----
            pv1 = psml.tile([P, NCH, 2 * H], BF, tag="s")
            pv2 = psml.tile([P, NCH, 2 * H], BF, tag="s")
            for c in range(NCH):
                nc.tensor.transpose(
                    pv1[:, c, :], uv1[:, c * P : c * P + P], ident[0 : 2 * H, 0 : 2 * H]
                )
                nc.tensor.transpose(
                    pv2[:, c, :], uv2[:, c * P : c * P + P], ident[0 : 2 * H, 0 : 2 * H]
                )
            vT1 = sg.tile([P, NCH, 2 * H], FP, tag="vT1")
            nc.vector.tensor_copy(vT1[:], pv1[:])
            vT2 = sg.tile([P, NCH, 2 * H], FP, tag="vT2")
            nc.vector.tensor_copy(vT2[:], pv2[:])

            hcat = sg.tile([P, NCH, HID], BF, tag="hcat")

            wrows = []
            for h in range(H):
                wrow = wrp.tile([1, N], BF, tag="wrow")
                nc.sync.dma_start(wrow[:], uv1[h : h + 1, :])
                wrows.append(wrow)

            for h in range(H):
                pwb = pbig.tile([P, N], FP, tag="b_mm")
                nc.tensor.matmul(pwb[:], ones_row[:], wrows[h][:], start=True, stop=True)
                wb = sh.tile([P, N], BF, tag="wb")
                nc.scalar.copy(wb[:], pwb[:])

                # Wh natural + ones col [128, 4, 65]
                whon = sh.tile([P, NCH, 65], BF, tag="whon")
                pn = pmed.tile([P, NCH, 64], BF, tag="m")
                po = 64 * (h % 2)
                for jc in range(NCH):
                    nc.tensor.transpose(
                        pn[:, jc, :],
                        whT[po : po + 64, h // 2, jc * P : jc * P + P],
                        ident[po : po + 64, po : po + 64],
                    )
                nc.scalar.copy(whon[:, :, 0:64], pn[:])
                nc.vector.memset(whon[:, :, 64], 1.0)

                # attention pT [128, 4, 512] bf16
                ptile = sh.tile([P, NCH, N], BF, tag="pt")
                for jc in range(NCH):
                    tmp = sh.tile([P, N], BF, tag="ptmp")
                    nc.vector.tensor_scalar(
                        tmp[:], wb[:],
                        vT1[:, jc, H + h : H + h + 1],
                        vT2[:, jc, H + h : H + h + 1],
                        AL.mult, AL.max,
                    )
                    nc.vector.tensor_tensor(
                        ptile[:, jc, :], tmp[:], maskT[:, jc, :], AL.mult
                    )

                # AV + rowsum -> normalize -> ELU -> hcat slice
                hn = sh.tile([P, NCH, 64], FP, tag="hn")
                pav = pmed.tile([P, NCH, 65], FP, tag="m")
                for ic in range(NCH):
                    for jc in range(NCH):
                        nc.tensor.matmul(
                            pav[:, ic, :],
                            ptile[:, jc, ic * P : ic * P + P],
                            whon[:, jc, :],
                            start=(jc == 0),
                            stop=(jc == NCH - 1),
                        )
                rcp = sh.tile([P, NCH], FP, tag="rcp")
                nc.vector.reciprocal(rcp[:], pav[:, :, 64:65])
                nc.vector.tensor_tensor(
                    hn[:], pav[:, :, 0:64],
                    rcp[:, :, None].to_broadcast((P, NCH, 64)), AL.mult,
                )
                ee = sh.tile([P, NCH, 64], FP, tag="ee")
                nc.scalar.activation(ee[:], hn[:], EXP)
                nc.vector.tensor_scalar(ee[:], ee[:], 1.0, -1.0, AL.min, AL.add)
                nc.vector.scalar_tensor_tensor(
                    hcat[:, :, 64 * h : 64 * h + 64], hn[:], 0.0, ee[:], AL.max, AL.add
                )

            # ---- layer 2 ----
            hcT = sg.tile([P, NCH, HID], BF, tag="hcT")
            for c in range(NCH):
                pt_ = ptp.tile([P, N], BF, tag="b_tp")
                for ic in range(NCH):
                    nc.tensor.transpose(
                        pt_[:, ic * P : ic * P + P],
                        hcat[:, ic, c * P : c * P + P],
                        ident[:],
                    )
                nc.scalar.copy(hcT[:, c, :], pt_[:])

            w2t = sg.tile([P, 2, N], BF, tag="w2t")
            for oc in range(2):
                pw = pbig.tile([P, N], FP, tag="b_mm")
                for hc in range(NCH):
                    nc.tensor.matmul(
                        pw[:],
                        w2b[:, hc, oc * P : oc * P + P],
                        hcT[:, hc, :],
                        start=(hc == 0),
                        stop=(hc == NCH - 1),
                    )
                nc.scalar.copy(w2t[:, oc, :], pw[:])

            pg = psml.tile([2, N], FP, tag="s")
            for oc in range(2):
                nc.tensor.matmul(
                    pg[:], a2b[:, oc, :], w2t[:, oc, :], start=(oc == 0), stop=(oc == 1)
                )
            uvA = sg.tile([2, N], BF, tag="uvA")
            nc.scalar.activation(uvA[:], pg[:], EXP, scale=sc2[:])
            uvB = sg.tile([2, N], BF, tag="uvB")
            nc.scalar.activation(uvB[:], pg[:], EXP, scale=ALPHA)

            pwb = pbig.tile([P, N], FP, tag="b_mm")
            nc.tensor.matmul(pwb[:], ones_row[:], uvA[0:1, :], start=True, stop=True)
            wb2 = sg.tile([P, N], BF, tag="wb2")
            nc.scalar.copy(wb2[:], pwb[:])

            pvA = psml.tile([P, NCH, 2], BF, tag="s")
            pvB = psml.tile([P, NCH, 2], BF, tag="s")
            for c in range(NCH):
                nc.tensor.transpose(pvA[:, c, :], uvA[:, c * P : c * P + P], ident[0:2, 0:2])
                nc.tensor.transpose(pvB[:, c, :], uvB[:, c * P : c * P + P], ident[0:2, 0:2])
            vA = sg.tile([P, NCH, 2], FP, tag="vA")
            nc.vector.tensor_copy(vA[:], pvA[:])
            vB = sg.tile([P, NCH, 2], FP, tag="vB")
            nc.vector.tensor_copy(vB[:], pvB[:])

            w2on = sg.tile([P, NCH, OUT + 1], BF, tag="w2on")
            for jc in range(NCH):
                p2n = pmed.tile([P, OUT], FP, tag="m")
                for hc in range(NCH):
                    nc.tensor.matmul(
                        p2n[:],
                        hcT[:, hc, jc * P : jc * P + P],
                        w2b[:, hc, :],
                        start=(hc == 0),
                        stop=(hc == NCH - 1),
                    )
                nc.scalar.copy(w2on[:, jc, 0:OUT], p2n[:])
            nc.vector.memset(w2on[:, :, OUT], 1.0)

            p2t = sg.tile([P, NCH, N], BF, tag="p2t")
            for jc in range(NCH):
                tmp = sh.tile([P, N], BF, tag="ptmp")
                nc.vector.tensor_scalar(
                    tmp[:], wb2[:], vA[:, jc, 1:2], vB[:, jc, 1:2], AL.mult, AL.max
                )
                nc.vector.tensor_tensor(p2t[:, jc, :], tmp[:], maskT[:, jc, :], AL.mult)

            hn2 = sg.tile([P, NCH, OUT], FP, tag="hn2")
            for ic in range(NCH):
                pav = pmed.tile([P, OUT + 1], FP, tag="m")
                for jc in range(NCH):
                    nc.tensor.matmul(
                        pav[:],
                        p2t[:, jc, ic * P : ic * P + P],
                        w2on[:, jc, :],
                        start=(jc == 0),
                        stop=(jc == NCH - 1),
                    )
                rcp = sh.tile([P, 1], FP, tag="rcp")
                nc.vector.reciprocal(rcp[:], pav[:, OUT : OUT + 1])
                nc.vector.tensor_scalar_mul(hn2[:, ic, :], pav[:, 0:OUT], rcp[:])

            ee2 = sg.tile([P, NCH, OUT], FP, tag="ee2")
            nc.scalar.activation(ee2[:], hn2[:], EXP)
            nc.vector.tensor_scalar(ee2[:], ee2[:], 1.0, -1.0, AL.min, AL.add)
            h2 = hn2
            nc.vector.scalar_tensor_tensor(h2[:], hn2[:], 0.0, ee2[:], AL.max, AL.add)

            # ---- LayerNorm over features ----
            sums = sg.tile([P, NCH], FP, tag="sums")
            nc.vector.reduce_sum(sums[:], h2[:], axis=mybir.AxisListType.X)
            nc.vector.tensor_scalar_mul(sums[:], sums[:], -1.0 / OUT)
            xc = h2
            nc.vector.tensor_tensor(
                xc[:], h2[:], sums[:, :, None].to_broadcast((P, NCH, OUT)), AL.add
            )
            ssq = sg.tile([P, NCH], FP, tag="ssq")
            for ic in range(NCH):
                nc.vector.scalar_tensor_tensor(
                    ee2[:, ic, :], xc[:, ic, :], 1.0, xc[:, ic, :], AL.mult, AL.mult,
                    accum_out=ssq[:, ic : ic + 1],
                )
            lnv = sg.tile([P, NCH], FP, tag="lnv")
            nc.scalar.activation(lnv[:], ssq[:], LN_F, bias=epst[:], scale=1.0 / OUT)
            rstd = sg.tile([P, NCH], FP, tag="rstd")
            nc.scalar.activation(rstd[:], lnv[:], EXP, scale=-0.5)
            ycast = sg.tile([P, NCH, OUT], BF, tag="ycast")
            for ic in range(NCH):
                nc.vector.scalar_tensor_tensor(
                    ycast[:, ic, :], xc[:, ic, :], rstd[:, ic : ic + 1], gamb[:],
                    AL.mult, AL.mult,
                )

            # ---- pooling via transpose + free-dim reduce ----
            ocol = sg.tile([P, 2], FP, tag="ocol")
            for oc in range(2):
                py = ptp.tile([P, N], BF, tag="b_tp")
                for ic in range(NCH):
                    nc.tensor.transpose(
                        py[:, ic * P : ic * P + P],
                        ycast[:, ic, oc * P : oc * P + P],
                        ident[:],
                    )
                sumr = sh.tile([P, 1], FP, tag="sumr")
                nc.vector.reduce_sum(sumr[:], py[:], axis=mybir.AxisListType.X)
                maxr = sh.tile([P, 1], FP, tag="maxr")
                nc.vector.reduce_max(maxr[:], py[:], axis=mybir.AxisListType.X)
                nc.vector.scalar_tensor_tensor(
                    ocol[:, oc : oc + 1], sumr[:], 1.0 / N, maxr[:], AL.mult, AL.add
                )
            nc.vector.tensor_tensor(ocol[:], ocol[:], b2c[:], AL.add)
            nc.sync.dma_start(out_d[g].rearrange("(c p) -> p c", p=P), ocol[:])

        for pool in (psml, ptp, pmed, pbig, sh, sg, wrp, gio, cp):
            pool.release()
    nc.compile()
    return nc


def _host_prep(inputs):
    """Pack weights to bf16 device layouts; pre-transpose x and support."""
    W1 = np.asarray(inputs["W1"], np.float32)
    a1s = np.asarray(inputs["a1s"], np.float32)
    a1d = np.asarray(inputs["a1d"], np.float32)
    W2 = np.asarray(inputs["W2"], np.float32)
    a2s = np.asarray(inputs["a2s"], np.float32)
    a2d = np.asarray(inputs["a2d"], np.float32)
    gamma = np.asarray(inputs["gamma"], np.float32)
    beta = np.asarray(inputs["beta"], np.float32)

    W1all = np.zeros((FC * P, HID), np.float32)
    W1all[:F] = np.transpose(W1, (1, 0, 2)).reshape(F, HID)
    w1b = np.ascontiguousarray(
        W1all.reshape(FC, P, HID).transpose(1, 0, 2)
    ).astype(BF_NP)

    a1b = np.zeros((P, 4, 2 * H), np.float32)
    for h in range(H):
        po = 64 * (h % 2)
        a1b[po : po + 64, h // 2, h] = a1s[h]
        a1b[po : po + 64, h // 2, H + h] = a1d[h]
    a1b = a1b.astype(BF_NP)

    w2b = np.ascontiguousarray(W2.reshape(4, P, OUT).transpose(1, 0, 2)).astype(BF_NP)
    a2b = np.stack(
        [a2s.reshape(2, P).T, a2d.reshape(2, P).T], axis=-1
    ).astype(BF_NP)  # [128, 2, 2]
    gamb = np.ascontiguousarray(np.broadcast_to(gamma[None, :], (P, OUT))).astype(
        np.float32
    )
    b2c = np.ascontiguousarray(2.0 * beta.reshape(2, P).T).astype(np.float32)

    x = np.asarray(inputs["x"], np.float32)
    sup = np.asarray(inputs["support"], np.float32)
    xpad = np.zeros((B, N, FC * P), np.float32)
    xpad[:, :, :F] = x
    xT = np.ascontiguousarray(
        xpad.reshape(B, N, FC, P).transpose(0, 3, 2, 1)
    ).astype(BF_NP)
    mT = np.ascontiguousarray(
        sup.reshape(B, N, NCH, P).transpose(0, 3, 2, 1)
    ).astype(BF_NP)
    return w1b, a1b, w2b, a2b, gamb, b2c, xT, mT


_NC_CACHE = None


def kernel(**inputs):
    global _NC_CACHE
    if _NC_CACHE is None:
        _NC_CACHE = build_kernel()
    nc = _NC_CACHE
    w1b, a1b, w2b, a2b, gamb, b2c, xT, mT = _host_prep(inputs)
    reps = {"w1b": w1b, "a1b": a1b, "w2b": w2b, "a2b": a2b, "gamb": gamb, "b2c": b2c}
    in_maps = []
    for c in range(NCORES):
        m = {
            "xT_sh": np.ascontiguousarray(xT[c * GPC : (c + 1) * GPC]),
            "mT_sh": np.ascontiguousarray(mT[c * GPC : (c + 1) * GPC]),
        }
        m.update(reps)
        in_maps.append(m)
    res = run_bass_kernel_spmd(nc, in_maps, core_ids=list(range(NCORES)))
    out = np.concatenate([r["out_sh"] for r in res.results], axis=0)
    return out.astype(np.float32)


if __name__ == "__main__":
    nc = build_kernel()
    print("built OK")


# revision 31
# speedup vs baseline: 1.0111x; 1.0111x over previous
"""GAT model kernel for Trainium2 — 8-core data-parallel over batch.

Per graph: 8-head GAT layer + single-head output attention + LayerNorm +
mean/max pooling. Attention exp uses the exact rank-1 factorization
  exp(leaky_relu(f1_i + f2_j)) = max(e^{f1+f2}, e^{a*(f1+f2)})
plus softmax row-scale invariance (divide row i by e^{a*f1_i}):
  p'[j,i] = mask[i,j] * max(w_i * v1_j, v2_j),
  w = e^{0.98 f1}, v1 = e^{f2}, v2 = e^{0.02 f2}
so only O(N) exps are needed per head instead of O(N^2).
Host side pre-packs weights to bf16 and pre-transposes x / support.
"""

import sys

sys.path.insert(0, "/opt/trn_rl_repo")

import ml_dtypes
import numpy as np

import concourse.bass as bass
from concourse import bacc
import concourse.mybir as mybir
import concourse.tile as tile
from concourse.bass_utils import run_bass_kernel_spmd

AL = mybir.AluOpType
FP = mybir.dt.float32
BF = mybir.dt.bfloat16
EXP = mybir.ActivationFunctionType.Exp
LN_F = mybir.ActivationFunctionType.Ln
RELU = mybir.ActivationFunctionType.Relu
BF_NP = ml_dtypes.bfloat16

B, N, F = 32, 512, 300
H, D = 8, 64
HID = H * D  # 512
OUT = 256
NCORES = 8
GPC = B // NCORES  # graphs per core
P = 128
NCH = N // P  # 4
FC = 3  # ceil(300/128)
ALPHA = 0.02
EPS = 1e-5


# ---- custom fused DVE op: out = max(in0*s0, s1) * in1 ----
from concourse import dve_ops as _dvo
from concourse.dve_spec import Spec as _Spec, Src0 as _S0, Src1 as _S1, C0 as _C0, C1 as _C1, maxx as _maxx, lower as _lower, _has_src1 as _has_src1
from concourse.dve_uop import DveOpSpec as _DveOpSpec


def _register_pmask():
    if "PMASK_ANT" in _dvo._SUB_OPCODE_FOR_NAME:
        return _dvo.OPS[_dvo._SUB_OPCODE_FOR_NAME["PMASK_ANT"] - _dvo._CUSTOM_DVE_ROW_BASE]
    spec = _Spec(
        body=_maxx(_S0 * _C0, _C1) * _S1,
        reference=lambda in0, in1, s0, s1, imm2: (
            np.maximum(in0.astype(np.float32) * s0, s1) * in1
        ).astype(np.float32),
    )
    shas = {}
    opcode = _dvo._CUSTOM_DVE_ROW_BASE + len(_dvo.OPS)
    for ver in ("v3", "v4"):
        r = _DveOpSpec(name="PMASK_ANT", opcode=opcode, uops=_lower(spec, ver=ver),
                       rd1_en=_has_src1(spec))
        shas[ver] = r.sha(ver)
    op = _dvo.DveOp("PMASK_ANT", spec, subdim=False, uops_sha=shas)
    _dvo.OPS.append(op)
    _dvo._SUB_OPCODE_FOR_NAME["PMASK_ANT"] = opcode
    _dvo.CUSTOM_DVE_SPECS["PMASK_ANT"] = spec
    return op


PMASK = _register_pmask()


def build_kernel():
    nc = bacc.Bacc()
    xT_d = nc.dram_tensor("xT_sh", [GPC, P, FC, N], BF, kind="ExternalInput").ap()
    mT_d = nc.dram_tensor("mT_sh", [GPC, P, NCH, N], BF, kind="ExternalInput").ap()
    w1_d = nc.dram_tensor("w1b", [P, FC, HID], BF, kind="ExternalInput").ap()
    a1_d = nc.dram_tensor("a1b", [P, 4, 2 * H], BF, kind="ExternalInput").ap()
    w2_d = nc.dram_tensor("w2b", [P, 4, OUT], BF, kind="ExternalInput").ap()
    a2_d = nc.dram_tensor("a2b", [P, 2, 2], BF, kind="ExternalInput").ap()
    gam_d = nc.dram_tensor("gamb", [P, OUT], FP, kind="ExternalInput").ap()
    b2_d = nc.dram_tensor("b2c", [P, 2], FP, kind="ExternalInput").ap()
    out_d = nc.dram_tensor("out_sh", [GPC, OUT], FP, kind="ExternalOutput").ap()

    ident_d = nc.inline_tensor(np.eye(P, dtype=np.float32).astype(BF_NP), name="idbf").ap()
    sc1_np = np.where(np.arange(2 * H) < H, 1.0 - ALPHA, 1.0).astype(np.float32)
    sc1_d = nc.inline_tensor(sc1_np[:, None], name="sc1c").ap()
    sc2_d = nc.inline_tensor(np.array([[1.0 - ALPHA], [1.0]], dtype=np.float32), name="sc2c").ap()

    with tile.TileContext(nc) as tc:
        cp = tc.alloc_tile_pool(name="const", bufs=1)
        gio = tc.alloc_tile_pool(name="gio", bufs=4)
        wrp = tc.alloc_tile_pool(name="wrp", bufs=2 * H)
        sg = tc.alloc_tile_pool(name="graph", bufs=2)
        sh = tc.alloc_tile_pool(name="head", bufs=3)
        pbig = tc.alloc_tile_pool(name="pbig", bufs=2, space="PSUM")
        pmed = tc.alloc_tile_pool(name="pmed", bufs=2, space="PSUM")
        ptp = tc.alloc_tile_pool(name="ptp", bufs=2, space="PSUM")
        psml = tc.alloc_tile_pool(name="psml", bufs=2, space="PSUM")

        # ---- constants (single clean DMAs) ----
        ident = cp.tile([P, P], BF, tag="ident")
        nc.sync.dma_start(ident[:], ident_d[:, :])
        ones_row = cp.tile([1, P], BF, tag="ones_row")
        nc.vector.memset(ones_row[:], 1.0)
        epst = cp.tile([P, 1], FP, tag="epst")
        nc.vector.memset(epst[:], EPS)
        w1b = cp.tile([P, FC, HID], BF, tag="w1b")
        nc.sync.dma_start(w1b[:], w1_d[:])
        a1b = cp.tile([P, 4, 2 * H], BF, tag="a1b")
        nc.sync.dma_start(a1b[:], a1_d[:])
        w2b = cp.tile([P, 4, OUT], BF, tag="w2b")
        nc.sync.dma_start(w2b[:], w2_d[:])
        a2b = cp.tile([P, 2, 2], BF, tag="a2b")
        nc.sync.dma_start(a2b[:], a2_d[:])
        gamb = cp.tile([P, OUT], FP, tag="gamb")
        nc.sync.dma_start(gamb[:], gam_d[:])
        b2c = cp.tile([P, 2], FP, tag="b2c")
        nc.sync.dma_start(b2c[:], b2_d[:])
        sc1 = cp.tile([2 * H, 1], FP, tag="sc1")
        nc.sync.dma_start(sc1[:], sc1_d[:, :])
        sc2 = cp.tile([2, 1], FP, tag="sc2")
        nc.sync.dma_start(sc2[:], sc2_d[:, :])

        for g in range(GPC):
            xT = gio.tile([P, FC, N], BF, tag="xT")
            nc.sync.dma_start(xT[:], xT_d[g])
            maskT = gio.tile([P, NCH, N], BF, tag="maskT")
            nc.sync.dma_start(maskT[:], mT_d[g])

            # ---- WhT_all [128, 4, 512] bf16 (rows = hid = h*64+d) ----
            whT = sg.tile([P, NCH, N], BF, tag="whT")
            for mc in range(NCH):
                pw = pbig.tile([P, N], FP, tag="b_mm")
                for kc in range(FC):
                    nc.tensor.matmul(
                        pw[:],
                        w1b[:, kc, mc * P : mc * P + P],
                        xT[:, kc, :],
                        start=(kc == 0),
                        stop=(kc == FC - 1),
                    )
                nc.scalar.copy(whT[:, mc, :], pw[:])

            # ---- f rows [16, 512]: 0-7 f1 per head, 8-15 f2 ----
            pf = psml.tile([2 * H, N], FP, tag="s")
            for c in range(NCH):
                nc.tensor.matmul(
                    pf[:], a1b[:, c, :], whT[:, c, :],
                    start=(c == 0), stop=(c == NCH - 1),
                )
            uv1 = sg.tile([2 * H, N], BF, tag="uv1")
            nc.scalar.activation(uv1[:], pf[:], EXP, scale=sc1[:])
            uv2 = sg.tile([2 * H, N], BF, tag="uv2")
            nc.scalar.activation(uv2[:], pf[:], EXP, scale=ALPHA)

            # ---- v columns via transpose: [128, 4, 16] ----
            pv1 = psml.tile([P, NCH, 2 * H], BF, tag="s")
            pv2 = psml.tile([P, NCH, 2 * H], BF, tag="s")
            for c in range(NCH):
                nc.tensor.transpose(
                    pv1[:, c, :], uv1[:, c * P : c * P + P], ident[0 : 2 * H, 0 : 2 * H]
                )
                nc.tensor.transpose(
                    pv2[:, c, :], uv2[:, c * P : c * P + P], ident[0 : 2 * H, 0 : 2 * H]
                )
            vT1 = sg.tile([P, NCH, 2 * H], FP, tag="vT1")
            nc.vector.tensor_copy(vT1[:], pv1[:])
            vT2 = sg.tile([P, NCH, 2 * H], FP, tag="vT2")
            nc.vector.tensor_copy(vT2[:], pv2[:])

            hcat = sg.tile([P, NCH, HID], BF, tag="hcat")

            wrows = []
            for h in range(H):
                wrow = wrp.tile([1, N], BF, tag="wrow")
                nc.sync.dma_start(wrow[:], uv1[h : h + 1, :])
                wrows.append(wrow)

            for h in range(H):
                pwb = pbig.tile([P, N], FP, tag="b_mm")
                nc.tensor.matmul(pwb[:], ones_row[:], wrows[h][:], start=True, stop=True)
                wb = sh.tile([P, N], BF, tag="wb")
                nc.scalar.copy(wb[:], pwb[:])

                # Wh natural + ones col [128, 4, 65]
                whon = sh.tile([P, NCH, 65], BF, tag="whon")
                pn = pmed.tile([P, NCH, 64], BF, tag="m")
                po = 64 * (h % 2)
                for jc in range(NCH):
                    nc.tensor.transpose(
                        pn[:, jc, :],
                        whT[po : po + 64, h // 2, jc * P : jc * P + P],
                        ident[po : po + 64, po : po + 64],
                    )
                nc.scalar.copy(whon[:, :, 0:64], pn[:])
                nc.vector.memset(whon[:, :, 64], 1.0)

                # attention pT [128, 4, 512] bf16
                ptile = sh.tile([P, NCH, N], BF, tag="pt")
                for jc in range(NCH):
                    tmp = sh.tile([P, N], BF, tag="ptmp")
                    nc.gpsimd.tensor_scalar(
                        tmp[:], wb[:],
                        vT1[:, jc, H + h : H + h + 1],
                        vT2[:, jc, H + h : H + h + 1],
                        AL.mult, AL.max,
                    )
                    nc.vector.tensor_tensor(
                        ptile[:, jc, :], tmp[:], maskT[:, jc, :], AL.mult
                    )

                # AV + rowsum -> normalize -> ELU -> hcat slice
                hn = sh.tile([P, NCH, 64], FP, tag="hn")
                pav = pmed.tile([P, NCH, 65], FP, tag="m")
                for ic in range(NCH):
                    for jc in range(NCH):
                        nc.tensor.matmul(
                            pav[:, ic, :],
                            ptile[:, jc, ic * P : ic * P + P],
                            whon[:, jc, :],
                            start=(jc == 0),
                            stop=(jc == NCH - 1),
                        )
                rcp = sh.tile([P, NCH], FP, tag="rcp")
                nc.vector.reciprocal(rcp[:], pav[:, :, 64:65])
                nc.vector.tensor_tensor(
                    hn[:], pav[:, :, 0:64],
                    rcp[:, :, None].to_broadcast((P, NCH, 64)), AL.mult,
                )
                ee = sh.tile([P, NCH, 64], FP, tag="ee")
                nc.scalar.activation(ee[:], hn[:], EXP)
                nc.vector.tensor_scalar(ee[:], ee[:], 1.0, -1.0, AL.min, AL.add)
                nc.vector.scalar_tensor_tensor(
                    hcat[:, :, 64 * h : 64 * h + 64], hn[:], 0.0, ee[:], AL.max, AL.add
                )

            # ---- layer 2 ----
            hcT = sg.tile([P, NCH, HID], BF, tag="hcT")
            for c in range(NCH):
                pt_ = ptp.tile([P, N], BF, tag="b_tp")
                for ic in range(NCH):
                    nc.tensor.transpose(
                        pt_[:, ic * P : ic * P + P],
                        hcat[:, ic, c * P : c * P + P],
                        ident[:],
                    )
                nc.scalar.copy(hcT[:, c, :], pt_[:])

            w2t = sg.tile([P, 2, N], BF, tag="w2t")
            for oc in range(2):
                pw = pbig.tile([P, N], FP, tag="b_mm")
                for hc in range(NCH):
                    nc.tensor.matmul(
                        pw[:],
                        w2b[:, hc, oc * P : oc * P + P],
                        hcT[:, hc, :],
                        start=(hc == 0),
                        stop=(hc == NCH - 1),
                    )
                nc.scalar.copy(w2t[:, oc, :], pw[:])

            pg = psml.tile([2, N], FP, tag="s")
            for oc in range(2):
                nc.tensor.matmul(
                    pg[:], a2b[:, oc, :], w2t[:, oc, :], start=(oc == 0), stop=(oc == 1)
                )
            uvA = sg.tile([2, N], BF, tag="uvA")
            nc.scalar.activation(uvA[:], pg[:], EXP, scale=sc2[:])
            uvB = sg.tile([2, N], BF, tag="uvB")
            nc.scalar.activation(uvB[:], pg[:], EXP, scale=ALPHA)

            pwb = pbig.tile([P, N], FP, tag="b_mm")
            nc.tensor.matmul(pwb[:], ones_row[:], uvA[0:1, :], start=True, stop=True)
            wb2 = sg.tile([P, N], BF, tag="wb2")
            nc.scalar.copy(wb2[:], pwb[:])

            pvA = psml.tile([P, NCH, 2], BF, tag="s")
            pvB = psml.tile([P, NCH, 2], BF, tag="s")
            for c in range(NCH):
                nc.tensor.transpose(pvA[:, c, :], uvA[:, c * P : c * P + P], ident[0:2, 0:2])
                nc.tensor.transpose(pvB[:, c, :], uvB[:, c * P : c * P + P], ident[0:2, 0:2])
            vA = sg.tile([P, NCH, 2], FP, tag="vA")
            nc.vector.tensor_copy(vA[:], pvA[:])
            vB = sg.tile([P, NCH, 2], FP, tag="vB")
            nc.vector.tensor_copy(vB[:], pvB[:])

            w2on = sg.tile([P, NCH, OUT + 1], BF, tag="w2on")
            for jc in range(NCH):
                p2n = pmed.tile([P, OUT], FP, tag="m")
                for hc in range(NCH):
                    nc.tensor.matmul(
                        p2n[:],
                        hcT[:, hc, jc * P : jc * P + P],
                        w2b[:, hc, :],
                        start=(hc == 0),
                        stop=(hc == NCH - 1),
                    )
                nc.scalar.copy(w2on[:, jc, 0:OUT], p2n[:])
            nc.vector.memset(w2on[:, :, OUT], 1.0)

            p2t = sg.tile([P, NCH, N], BF, tag="p2t")
            for jc in range(NCH):
                tmp = sh.tile([P, N], BF, tag="ptmp")
                nc.gpsimd.tensor_scalar(
                    tmp[:], wb2[:], vA[:, jc, 1:2], vB[:, jc, 1:2], AL.mult, AL.max
                )
                nc.vector.tensor_tensor(p2t[:, jc, :], tmp[:], maskT[:, jc, :], AL.mult)

            hn2 = sg.tile([P, NCH, OUT], FP, tag="hn2")
            for ic in range(NCH):
                pav = pmed.tile([P, OUT + 1], FP, tag="m")
                for jc in range(NCH):
                    nc.tensor.matmul(
                        pav[:],
                        p2t[:, jc, ic * P : ic * P + P],
                        w2on[:, jc, :],
                        start=(jc == 0),
                        stop=(jc == NCH - 1),
                    )
                rcp = sh.tile([P, 1], FP, tag="rcp")
                nc.vector.reciprocal(rcp[:], pav[:, OUT : OUT + 1])
                nc.vector.tensor_scalar_mul(hn2[:, ic, :], pav[:, 0:OUT], rcp[:])

            ee2 = sg.tile([P, NCH, OUT], FP, tag="ee2")
            nc.scalar.activation(ee2[:], hn2[:], EXP)
            nc.vector.tensor_scalar(ee2[:], ee2[:], 1.0, -1.0, AL.min, AL.add)
            h2 = hn2
            nc.vector.scalar_tensor_tensor(h2[:], hn2[:], 0.0, ee2[:], AL.max, AL.add)

            # ---- LayerNorm over features ----
            sums = sg.tile([P, NCH], FP, tag="sums")
            nc.vector.reduce_sum(sums[:], h2[:], axis=mybir.AxisListType.X)
            nc.vector.tensor_scalar_mul(sums[:], sums[:], -1.0 / OUT)
            xc = h2
            nc.vector.tensor_tensor(
                xc[:], h2[:], sums[:, :, None].to_broadcast((P, NCH, OUT)), AL.add
            )
            ssq = sg.tile([P, NCH], FP, tag="ssq")
            for ic in range(NCH):
                nc.vector.scalar_tensor_tensor(
                    ee2[:, ic, :], xc[:, ic, :], 1.0, xc[:, ic, :], AL.mult, AL.mult,
                    accum_out=ssq[:, ic : ic + 1],
                )
            lnv = sg.tile([P, NCH], FP, tag="lnv")
            nc.scalar.activation(lnv[:], ssq[:], LN_F, bias=epst[:], scale=1.0 / OUT)
            rstd = sg.tile([P, NCH], FP, tag="rstd")
            nc.scalar.activation(rstd[:], lnv[:], EXP, scale=-0.5)
            ycast = sg.tile([P, NCH, OUT], BF, tag="ycast")
            for ic in range(NCH):
                nc.vector.scalar_tensor_tensor(
                    ycast[:, ic, :], xc[:, ic, :], rstd[:, ic : ic + 1], gamb[:],
                    AL.mult, AL.mult,
                )

            # ---- pooling via transpose + free-dim reduce ----
            ocol = sg.tile([P, 2], FP, tag="ocol")
            for oc in range(2):
                py = ptp.tile([P, N], BF, tag="b_tp")
                for ic in range(NCH):
                    nc.tensor.transpose(
                        py[:, ic * P : ic * P + P],
                        ycast[:, ic, oc * P : oc * P + P],
                        ident[:],
                    )
                sumr = sh.tile([P, 1], FP, tag="sumr")
                nc.vector.reduce_sum(sumr[:], py[:], axis=mybir.AxisListType.X)
                maxr = sh.tile([P, 1], FP, tag="maxr")
                nc.vector.reduce_max(maxr[:], py[:], axis=mybir.AxisListType.X)
                nc.vector.scalar_tensor_tensor(
                    ocol[:, oc : oc + 1], sumr[:], 1.0 / N, maxr[:], AL.mult, AL.add
                )
            nc.vector.tensor_tensor(ocol[:], ocol[:], b2c[:], AL.add)
            nc.sync.dma_start(out_d[g].rearrange("(c p) -> p c", p=P), ocol[:])

        for pool in (psml, ptp, pmed, pbig, sh, sg, wrp, gio, cp):
            pool.release()
    nc.compile()
    return nc


def _host_prep(inputs):
    """Pack weights to bf16 device layouts; pre-transpose x and support."""
    W1 = np.asarray(inputs["W1"], np.float32)
    a1s = np.asarray(inputs["a1s"], np.float32)
    a1d = np.asarray(inputs["a1d"], np.float32)
    W2 = np.asarray(inputs["W2"], np.float32)
    a2s = np.asarray(inputs["a2s"], np.float32)
    a2d = np.asarray(inputs["a2d"], np.float32)
    gamma = np.asarray(inputs["gamma"], np.float32)
    beta = np.asarray(inputs["beta"], np.float32)

    W1all = np.zeros((FC * P, HID), np.float32)
    W1all[:F] = np.transpose(W1, (1, 0, 2)).reshape(F, HID)
    w1b = np.ascontiguousarray(
        W1all.reshape(FC, P, HID).transpose(1, 0, 2)
    ).astype(BF_NP)

    a1b = np.zeros((P, 4, 2 * H), np.float32)
    for h in range(H):
        po = 64 * (h % 2)
        a1b[po : po + 64, h // 2, h] = a1s[h]
        a1b[po : po + 64, h // 2, H + h] = a1d[h]
    a1b = a1b.astype(BF_NP)

    w2b = np.ascontiguousarray(W2.reshape(4, P, OUT).transpose(1, 0, 2)).astype(BF_NP)
    a2b = np.stack(
        [a2s.reshape(2, P).T, a2d.reshape(2, P).T], axis=-1
    ).astype(BF_NP)  # [128, 2, 2]
    gamb = np.ascontiguousarray(np.broadcast_to(gamma[None, :], (P, OUT))).astype(
        np.float32
    )
    b2c = np.ascontiguousarray(2.0 * beta.reshape(2, P).T).astype(np.float32)

    x = np.asarray(inputs["x"], np.float32)
    sup = np.asarray(inputs["support"], np.float32)
    xpad = np.zeros((B, N, FC * P), np.float32)
    xpad[:, :, :F] = x
    xT = np.ascontiguousarray(
        xpad.reshape(B, N, FC, P).transpose(0, 3, 2, 1)
    ).astype(BF_NP)
    mT = np.ascontiguousarray(
        sup.reshape(B, N, NCH, P).transpose(0, 3, 2, 1)
    ).astype(BF_NP)
    return w1b, a1b, w2b, a2b, gamb, b2c, xT, mT


_NC_CACHE = None


def kernel(**inputs):
    global _NC_CACHE
    if _NC_CACHE is None:
        _NC_CACHE = build_kernel()
    nc = _NC_CACHE
    w1b, a1b, w2b, a2b, gamb, b2c, xT, mT = _host_prep(inputs)
    reps = {"w1b": w1b, "a1b": a1b, "w2b": w2b, "a2b": a2b, "gamb": gamb, "b2c": b2c}
    in_maps = []
    for c in range(NCORES):
        m = {
            "xT_sh": np.ascontiguousarray(xT[c * GPC : (c + 1) * GPC]),
            "mT_sh": np.ascontiguousarray(mT[c * GPC : (c + 1) * GPC]),
        }
        m.update(reps)
        in_maps.append(m)
    res = run_bass_kernel_spmd(nc, in_maps, core_ids=list(range(NCORES)))
    out = np.concatenate([r["out_sh"] for r in res.results], axis=0)
    return out.astype(np.float32)


if __name__ == "__main__":
    nc = build_kernel()
    print("built OK")
